# revision 2
# baseline (speedup 1.0000x reference)
"""3-layer GCN (DGL GraphConv norm='both') on 8 TRN2 NeuronCores.

Strategy (edge-cut, dst-owner sharding):
  - Permute the 100k nodes into 960 degree-balanced slices of 128 dst slots
    (120 slices per core; a core owns a contiguous 15360-slot block).
  - Node features live replicated per core in DRAM ("xn" tables, pre-scaled
    by the src-side degree norm). The table is viewed as 4 chunks of 30720
    rows so gather indices fit in int16 for dma_gather (<= 32767).
  - Per (slice, chunk) the incident edges are padded to 512 gather slots
    (pad: idx 0 / dstrel -1); a gather call covers a slice PAIR (1024 idx,
    the dma_gather per-call limit). The gathered layout puts edge j in
    partition j%128, block j//128 -- exactly matmul lhsT tiles.
  - Segment-sum via one-hot S_T (DVE tensor_scalar is_equal vs an iota
    constant) and PE matmuls accumulated in PSUM:
        acc[fi, slot] += gathered_tile.T @ S_T
  - Per slice: out[slot, fo] = acc.T @ W via a second matmul (acc is already
    [fi, slot] so lhsT needs no transpose), then fused relu(out * norm) on
    the scalar engine. Layers 1-2 fold the next layer's src-norm into the
    same scale; layer 3 adds the bias.
  - An 8-core AllGather rebuilds the replicated xn table between layers.
"""

import os as _os

import numpy as np

import concourse.bass as bass
import concourse.mybir as mybir
import concourse.tile as tile
from concourse import bacc, library_config
from concourse.bass_utils import run_bass_kernel_spmd

P = 128
NCORES = 8
N_NODES = 100000
SLICES_TOTAL = 960
SLICES_CORE = SLICES_TOTAL // NCORES  # 120
N_PAD = SLICES_TOTAL * P  # 122880
PER_CORE = SLICES_CORE * P  # 15360
NCHUNK = 4
CHUNK_ROWS = N_PAD // NCHUNK  # 30720 < 32768
TSC = 4  # gather tiles per (slice, chunk)
SLOT_CAP = TSC * P  # 512 edge slots per (slice, chunk)
NPAIR = SLICES_CORE // 2  # 60 slice pairs
NI_CALL = 2 * SLOT_CAP  # 1024 = dma_gather per-call limit
F_IN = 128
F_HID = 128
F_OUT = 64

N_TILE_COLS = SLICES_CORE * NCHUNK * TSC  # 1920 dstrel columns
IDX_COLS = SLICES_CORE * NCHUNK * SLOT_CAP // 16  # 15360 idx columns

_DBG_LAYERS = int(_os.environ.get("GNN_LAYERS", "3"))
_DBG_PAIRS = int(_os.environ.get("GNN_PAIRS", str(NPAIR)))
_DBG_NO_AG = int(_os.environ.get("GNN_NO_AG", "0"))
_DBG_REPS = int(_os.environ.get("GNN_REPS", "1"))
_SKIP_GATHER = int(_os.environ.get("GNN_SKIP_GATHER", "0"))
_SKIP_COMPUTE = int(_os.environ.get("GNN_SKIP_COMPUTE", "0"))


def _preprocess(src, dst):
    """Permutation + per-bucket padded gather indices / dst slots."""
    n = N_NODES
    deg_out = np.bincount(src, minlength=n).astype(np.float32)
    deg_in = np.bincount(dst, minlength=n).astype(np.float32)
    ns = 1.0 / np.sqrt(np.maximum(deg_out, 1.0))
    nd = 1.0 / np.sqrt(np.maximum(deg_in, 1.0))

    # Degree-balanced slice assignment: snake round-robin over slices in
    # descending in-degree order. perm[node] = slice*128 + slot.
    order = np.argsort(-deg_in, kind="stable")
    slice_of = np.empty(n, dtype=np.int64)
    slot_of = np.empty(n, dtype=np.int64)
    for k in range(0, n, SLICES_TOTAL):
        stratum = order[k : k + SLICES_TOTAL]
        slot = k // SLICES_TOTAL
        m = len(stratum)
        if (slot % 2) == 0:
            slices = np.arange(m)
        else:
            slices = SLICES_TOTAL - 1 - np.arange(m)
        slice_of[stratum] = slices
        slot_of[stratum] = slot
    perm = slice_of * P + slot_of

    s_perm = perm[src]
    d_perm = perm[dst]
    chunk = s_perm // CHUNK_ROWS
    idxval = (s_perm % CHUNK_ROWS).astype(np.int16)
    slot = (d_perm % P).astype(np.float32)
    key = (d_perm // P) * NCHUNK + chunk  # (global slice, chunk) bucket

    nbuckets = SLICES_TOTAL * NCHUNK
    counts = np.bincount(key, minlength=nbuckets)
    if counts.max() > SLOT_CAP:
        raise RuntimeError(f"bucket overflow: {counts.max()} > {SLOT_CAP}")
    eorder = np.argsort(key, kind="stable")
    offs = np.zeros(nbuckets + 1, dtype=np.int64)
    np.cumsum(counts, out=offs[1:])
    pos = np.arange(len(src)) - offs[key[eorder]]

    idx_pad = np.zeros((nbuckets, SLOT_CAP), dtype=np.int16)
    rel_pad = np.full((nbuckets, SLOT_CAP), -1.0, dtype=np.float32)
    idx_pad[key[eorder], pos] = idxval[eorder]
    rel_pad[key[eorder], pos] = slot[eorder]

    return perm, ns, nd, idx_pad, rel_pad


def _wrap16(flat):
    """[NI] int16 -> [128, NI//16]: element j at [j%16, j//16], replicated x8."""
    w = flat.reshape(-1, 16).T
    return np.tile(w, (8, 1))


def _core_arrays(core, perm, ns, nd, idx_pad, rel_pad):
    """Per-core idx/dstrel/norm arrays matching the device loop order."""
    idx_all = np.empty((P, IDX_COLS), dtype=np.int16)
    rel_cols = np.empty((P, N_TILE_COLS), dtype=np.float32)
    icol = 0
    col = 0
    base_slice = core * SLICES_CORE
    for pr in range(NPAIR):
        for c in range(NCHUNK):
            flat = np.concatenate(
                [idx_pad[(base_slice + 2 * pr + si) * NCHUNK + c] for si in range(2)]
            )
            idx_all[:, icol : icol + NI_CALL // 16] = _wrap16(flat)
            icol += NI_CALL // 16
        for si in range(2):
            for c in range(NCHUNK):
                blk = rel_pad[(base_slice + 2 * pr + si) * NCHUNK + c]
                rel_cols[:, col : col + TSC] = blk.reshape(TSC, P).T
                col += TSC
    assert icol == IDX_COLS and col == N_TILE_COLS

    nds12 = np.zeros((P, SLICES_CORE), dtype=np.float32)
    nd3 = np.zeros((P, SLICES_CORE), dtype=np.float32)
    base = core * PER_CORE
    mask = (perm >= base) & (perm < base + PER_CORE)
    local = perm[mask] - base
    nds12[local % P, local // P] = (nd * ns)[mask]
    nd3[local % P, local // P] = nd[mask]
    return idx_all, rel_cols, nds12, nd3


def _build_program():
    nc = bacc.Bacc("TRN2", target_bir_lowering=False, debug=False, num_devices=NCORES, num_swdge_queues=4)
    dt = mybir.dt

    xn0_in = nc.declare_dram_parameter("xn0", [N_PAD, F_IN], dt.float32, isOutput=False)
    idx_in = nc.declare_dram_parameter("idx", [P, IDX_COLS], dt.int16, isOutput=False)
    rel_in = nc.declare_dram_parameter("rel", [P, N_TILE_COLS], dt.float32, isOutput=False)
    iota_in = nc.declare_dram_parameter("iota", [P, P], dt.float32, isOutput=False)
    w1_in = nc.declare_dram_parameter("w1", [F_IN, F_HID], dt.float32, isOutput=False)
    w2_in = nc.declare_dram_parameter("w2", [F_HID, F_HID], dt.float32, isOutput=False)
    w3_in = nc.declare_dram_parameter("w3", [F_HID, F_OUT], dt.float32, isOutput=False)
    b3_in = nc.declare_dram_parameter("b3rep", [P, F_OUT], dt.float32, isOutput=False)
    nds12_in = nc.declare_dram_parameter("nds12", [P, SLICES_CORE], dt.float32, isOutput=False)
    nd3_in = nc.declare_dram_parameter("nd3", [P, SLICES_CORE], dt.float32, isOutput=False)
    out_ext = nc.declare_dram_parameter("out", [PER_CORE, F_OUT], dt.float32, isOutput=True)

    with tile.TileContext(nc) as tc:
        with (
            tc.tile_pool(name="consts", bufs=1) as consts,
            tc.tile_pool(name="gt", bufs=4) as gtp,
            tc.tile_pool(name="work", bufs=8) as work,
            tc.tile_pool(name="outw", bufs=3) as outw,
            tc.tile_pool(name="psum", bufs=6, space="PSUM") as psum,
            tc.tile_pool(name="psw", bufs=2, space="PSUM") as psw,
            tc.tile_pool(name="dram", bufs=1, space="DRAM") as dram,
        ):
            nc.gpsimd.load_library(library_config.mlp)

            idx_t = consts.tile([P, IDX_COLS], dt.int16)
            rel_t = consts.tile([P, N_TILE_COLS], dt.float32)
            iota_t = consts.tile([P, P], dt.float32)
            w1_t = consts.tile([F_IN, F_HID], dt.float32)
            w2_t = consts.tile([F_HID, F_HID], dt.float32)
            w3_t = consts.tile([F_HID, F_OUT], dt.float32)
            b3_t = consts.tile([P, F_OUT], dt.float32)
            nds12_t = consts.tile([P, SLICES_CORE], dt.float32)
            nd3_t = consts.tile([P, SLICES_CORE], dt.float32)
            nc.sync.dma_start(out=idx_t[:], in_=idx_in[:])
            nc.sync.dma_start(out=rel_t[:], in_=rel_in[:])
            nc.sync.dma_start(out=iota_t[:], in_=iota_in[:])
            nc.sync.dma_start(out=w1_t[:], in_=w1_in[:])
            nc.sync.dma_start(out=w2_t[:], in_=w2_in[:])
            nc.sync.dma_start(out=w3_t[:], in_=w3_in[:])
            nc.sync.dma_start(out=b3_t[:], in_=b3_in[:])
            nc.sync.dma_start(out=nds12_t[:], in_=nds12_in[:])
            nc.sync.dma_start(out=nd3_t[:], in_=nd3_in[:])

            ag_in1 = dram.tile([PER_CORE, F_HID], dt.float32, tag="ag_in1")
            ag_in2 = dram.tile([PER_CORE, F_HID], dt.float32, tag="ag_in2")
            xn1 = dram.tile([N_PAD, F_HID], dt.float32, tag="xn1")
            xn2 = dram.tile([N_PAD, F_HID], dt.float32, tag="xn2")

            layers = [
                (xn0_in, w1_t, F_HID, nds12_t, ag_in1, xn1),
                (xn1, w2_t, F_HID, nds12_t, ag_in2, xn2),
                (xn2, w3_t, F_OUT, nd3_t, None, None),
            ]

            tc._gnn = (gtp, work, outw, psum, psw, idx_t, rel_t, iota_t, b3_t, out_ext)
            import contextlib

            loop_cm = (
                tc.For_i(0, _DBG_REPS, 1)
                if _DBG_REPS > 1
                else contextlib.nullcontext()
            )
            with loop_cm:
                _emit_layers(nc, tc, layers)
    nc.compile()
    return nc


def _emit_layers(nc, tc, layers):
    dt = mybir.dt
    gtp, work, outw, psum, psw, idx_t, rel_t, iota_t, b3_t, out_ext = tc._gnn
    if True:
        if True:
            for li, (table, w_t, fo, scale_t, ag_in, ag_out) in enumerate(layers):
                if li >= _DBG_LAYERS:
                    break
                with nc.named_scope(f"layer{li + 1}"):
                    icol = 0
                    col = 0
                    for pr in range(NPAIR):
                        if pr >= _DBG_PAIRS:
                            break
                        gts = []
                        for c in range(NCHUNK):
                            gt = gtp.tile([P, 2 * TSC, P], dt.float32, tag=f"gt{c}")
                            if _SKIP_GATHER:
                                nc.gpsimd.memset(gt[:, 0:1, 0:4], 0)
                            if not _SKIP_GATHER:
                                nc.gpsimd.dma_gather(
                                    gt[:],
                                    table[c * CHUNK_ROWS : (c + 1) * CHUNK_ROWS, :],
                                    idx_t[:, icol : icol + NI_CALL // 16],
                                    NI_CALL,
                                    NI_CALL,
                                    P,
                                    queue_num=c,
                                )
                            icol += NI_CALL // 16
                            gts.append(gt)
                        for si in range(2):
                            if _SKIP_COMPUTE:
                                break
                            s = 2 * pr + si
                            acc = psum.tile([P, P], dt.float32, space="PSUM", tag="acc")
                            for c in range(NCHUNK):
                                for t in range(TSC):
                                    s_t = work.tile([P, P], dt.float32, tag="s_t")
                                    nc.vector.tensor_scalar(
                                        out=s_t[:],
                                        in0=iota_t[:],
                                        scalar1=rel_t[:, col : col + 1],
                                        scalar2=None,
                                        op0=mybir.AluOpType.is_equal,
                                    )
                                    nc.tensor.matmul(
                                        out=acc[:],
                                        lhsT=gts[c][:, si * TSC + t, :],
                                        rhs=s_t[:],
                                        start=(c == 0 and t == 0),
                                        stop=(c == NCHUNK - 1 and t == TSC - 1),
                                    )
                                    col += 1
                            aggT = work.tile([P, P], dt.float32, tag="aggT")
                            nc.vector.tensor_copy(out=aggT[:], in_=acc[:])
                            op = psw.tile([P, fo], dt.float32, space="PSUM", tag="op")
                            nc.tensor.matmul(
                                out=op[:], lhsT=aggT[:], rhs=w_t[:], start=True, stop=True
                            )
                            o = outw.tile([P, fo], dt.float32, tag="o")
                            if li < 2:
                                nc.scalar.activation(
                                    out=o[:],
                                    in_=op[:],
                                    func=mybir.ActivationFunctionType.Relu,
                                    scale=scale_t[:, s : s + 1],
                                )
                                nc.sync.dma_start(
                                    out=ag_in[s * P : (s + 1) * P, :], in_=o[:]
                                )
                            else:
                                nc.scalar.activation(
                                    out=o[:],
                                    in_=op[:],
                                    func=mybir.ActivationFunctionType.Copy,
                                    scale=scale_t[:, s : s + 1],
                                )
                                nc.vector.tensor_add(out=o[:], in0=o[:], in1=b3_t[:])
                                nc.sync.dma_start(
                                    out=out_ext[s * P : (s + 1) * P, :], in_=o[:]
                                )
                    if ag_in is not None and not _DBG_NO_AG and _DBG_REPS == 1:
                        nc.gpsimd.collective_compute(
                            "AllGather",
                            mybir.AluOpType.bypass,
                            replica_groups=[list(range(NCORES))],
                            ins=[ag_in.opt()],
                            outs=[ag_out.opt()],
                        )


def _make_in_maps(x, src, dst, W1, W2, W3, b3):
    perm, ns, nd, idx_pad, rel_pad = _preprocess(src, dst)

    xn0 = np.zeros((N_PAD, F_IN), dtype=np.float32)
    xn0[perm] = x * ns[:, None]
    iota = np.broadcast_to(np.arange(P, dtype=np.float32), (P, P)).copy()
    b3rep = np.broadcast_to(b3, (P, F_OUT)).copy()

    in_maps = []
    for c in range(NCORES):
        idx_all, rel_cols, nds12, nd3 = _core_arrays(c, perm, ns, nd, idx_pad, rel_pad)
        in_maps.append(
            {
                "xn0": xn0,
                "idx": idx_all,
                "rel": rel_cols,
                "iota": iota,
                "w1": W1,
                "w2": W2,
                "w3": W3,
                "b3rep": b3rep,
                "nds12": nds12,
                "nd3": nd3,
            }
        )
    return in_maps, perm


def kernel(x, src, dst, W1, W2, W3, b3):
    x = np.ascontiguousarray(np.asarray(x, dtype=np.float32))
    src = np.asarray(src).astype(np.int64)
    dst = np.asarray(dst).astype(np.int64)
    W1 = np.ascontiguousarray(np.asarray(W1, dtype=np.float32))
    W2 = np.ascontiguousarray(np.asarray(W2, dtype=np.float32))
    W3 = np.ascontiguousarray(np.asarray(W3, dtype=np.float32))
    b3 = np.ascontiguousarray(np.asarray(b3, dtype=np.float32))

    in_maps, perm = _make_in_maps(x, src, dst, W1, W2, W3, b3)
    nc = _build_program()
    res = run_bass_kernel_spmd(nc, in_maps, list(range(NCORES)))
    global LAST_RESULT
    LAST_RESULT = res

    full = np.concatenate([res.results[c]["out"] for c in range(NCORES)], axis=0)
    return full[perm].astype(np.float32)


LAST_RESULT = None



# revision 7
# speedup vs baseline: 2.2889x; 2.2889x over previous
"""3-layer GCN (DGL GraphConv norm='both') on 8 TRN2 NeuronCores.

Strategy (edge-cut, dst-owner sharding), v2:
  - Permute the 100k nodes into 960 degree-balanced slices of 128 dst slots
    (120 slices per core; a core owns a contiguous 15360-slot block).
  - Node features live replicated per core in DRAM ("xn" tables, bf16,
    pre-scaled by the src-side degree norm). Table rows are laid out
    quartile-major: row = k*30720 + core*3840 + (s_loc%30)*128 + slot with
    k = s_loc//30, so each 30720-row chunk (int16 gather-index range) is
    exactly the output of one contiguous 8-core sub-AllGather.
  - Per (dst slice, chunk) the incident edges are padded to 512 gather slots
    (pad: idx 0 / dstrel -1); one dma_gather call covers a GROUP of 10
    slices x one chunk (5120 idx) to amortize the ~1us SWDGE fixed cost.
    The gathered layout puts edge j in partition j%128, block j//128 --
    exactly matmul lhsT tiles.
  - Segment-sum via one-hot S_T built with a SINGLE broadcast-AP DVE
    tensor_tensor is_equal per slice ([128, 16*128] bf16), then 16 PE
    matmuls accumulated in PSUM: acc[fi, slot] += gathered_tile.T @ S_T.
  - Per slice: out[slot, fo] = acc.T @ W via a second matmul, then fused
    relu(out * norm) on the scalar engine (bf16 out). Layers 1-2 fold the
    next layer's src-norm into the same scale; layer 3 adds the bias (fp32).
  - Between layers, 4 contiguous bf16 sub-AllGathers (one per quartile,
    Shared output) rebuild the replicated table; sub-AG k only depends on
    local slices [30k, 30k+30), so 3 of 4 overlap with remaining compute.
"""

import os as _os

import ml_dtypes
import numpy as np

import concourse.bass as bass
import concourse.mybir as mybir
import concourse.tile as tile
from concourse import bacc, library_config
from concourse.bass_utils import run_bass_kernel_spmd

P = 128
NCORES = 8
N_NODES = 100000
SLICES_TOTAL = 960
SLICES_CORE = SLICES_TOTAL // NCORES  # 120
N_PAD = SLICES_TOTAL * P  # 122880
PER_CORE = SLICES_CORE * P  # 15360
NCHUNK = 4
CHUNK_ROWS = N_PAD // NCHUNK  # 30720 < 32768
QUART = SLICES_CORE // NCHUNK  # 30 local slices per quartile
TSC = 4  # gather tiles per (slice, chunk)
SLOT_CAP = TSC * P  # 512 edge slots per (slice, chunk)
GRP = 2  # slices per gather call (1024-idx dma_gather per-call hard limit)
NGROUP = SLICES_CORE // GRP  # 60
NI_CALL = GRP * SLOT_CAP  # 5120 idx per dma_gather call
F_IN = 128
F_HID = 128
F_OUT = 64
NBLK = NCHUNK * TSC  # 16 col blocks per slice

N_TILE_COLS = SLICES_CORE * NBLK  # 1920 dstrel columns
IDX_COLS = SLICES_CORE * NCHUNK * SLOT_CAP // 16  # 15360 idx columns

_DBG_LAYERS = int(_os.environ.get("GNN_LAYERS", "3"))
_DBG_GROUPS = int(_os.environ.get("GNN_GROUPS", str(NGROUP)))
_DBG_NO_AG = int(_os.environ.get("GNN_NO_AG", "0"))

BF16 = ml_dtypes.bfloat16


def _preprocess(src, dst):
    """Permutation + per-bucket padded gather indices / dst slots."""
    n = N_NODES
    deg_out = np.bincount(src, minlength=n).astype(np.float32)
    deg_in = np.bincount(dst, minlength=n).astype(np.float32)
    ns = 1.0 / np.sqrt(np.maximum(deg_out, 1.0))
    nd = 1.0 / np.sqrt(np.maximum(deg_in, 1.0))

    # Degree-balanced slice assignment: snake round-robin over slices in
    # descending in-degree order. perm[node] = slice*128 + slot.
    order = np.argsort(-deg_in, kind="stable")
    slice_of = np.empty(n, dtype=np.int64)
    slot_of = np.empty(n, dtype=np.int64)
    for k in range(0, n, SLICES_TOTAL):
        stratum = order[k : k + SLICES_TOTAL]
        slot = k // SLICES_TOTAL
        m = len(stratum)
        if (slot % 2) == 0:
            slices = np.arange(m)
        else:
            slices = SLICES_TOTAL - 1 - np.arange(m)
        slice_of[stratum] = slices
        slot_of[stratum] = slot
    perm = slice_of * P + slot_of

    # Table row layout: quartile-major so chunk k == sub-AllGather k output.
    g_s = perm // P
    slot = perm % P
    core = g_s // SLICES_CORE
    s_loc = g_s % SLICES_CORE
    k = s_loc // QUART
    trow = k * CHUNK_ROWS + core * (QUART * P) + (s_loc % QUART) * P + slot

    s_row = trow[src]
    d_perm = perm[dst]
    chunk = s_row // CHUNK_ROWS
    idxval = (s_row % CHUNK_ROWS).astype(np.int16)
    dslot = (d_perm % P).astype(np.float32)
    key = (d_perm // P) * NCHUNK + chunk  # (global dst slice, src chunk)

    nbuckets = SLICES_TOTAL * NCHUNK
    counts = np.bincount(key, minlength=nbuckets)
    if counts.max() > SLOT_CAP:
        raise RuntimeError(f"bucket overflow: {counts.max()} > {SLOT_CAP}")
    eorder = np.argsort(key, kind="stable")
    offs = np.zeros(nbuckets + 1, dtype=np.int64)
    np.cumsum(counts, out=offs[1:])
    pos = np.arange(len(src)) - offs[key[eorder]]

    idx_pad = np.zeros((nbuckets, SLOT_CAP), dtype=np.int16)
    rel_pad = np.full((nbuckets, SLOT_CAP), -1.0, dtype=np.float32)
    idx_pad[key[eorder], pos] = idxval[eorder]
    rel_pad[key[eorder], pos] = dslot[eorder]

    return perm, trow, ns, nd, idx_pad, rel_pad


def _wrap16(flat):
    """[NI] int16 -> [128, NI//16]: element j at [j%16, j//16], replicated x8."""
    w = flat.reshape(-1, 16).T
    return np.tile(w, (8, 1))


def _core_arrays(core, perm, ns, nd, idx_pad, rel_pad):
    """Per-core idx/dstrel/norm arrays matching the device loop order."""
    idx_all = np.empty((P, IDX_COLS), dtype=np.int16)
    rel_cols = np.empty((P, N_TILE_COLS), dtype=BF16)
    icol = 0
    base_slice = core * SLICES_CORE
    for g in range(NGROUP):
        for c in range(NCHUNK):
            flat = np.concatenate(
                [
                    idx_pad[(base_slice + g * GRP + si) * NCHUNK + c]
                    for si in range(GRP)
                ]
            )
            idx_all[:, icol : icol + NI_CALL // 16] = _wrap16(flat)
            icol += NI_CALL // 16
    for s in range(SLICES_CORE):
        for c in range(NCHUNK):
            blk = rel_pad[(base_slice + s) * NCHUNK + c]  # [512]
            cols = blk.reshape(TSC, P).T.astype(BF16)  # [128, 4]
            rel_cols[:, s * NBLK + c * TSC : s * NBLK + c * TSC + TSC] = cols
    assert icol == IDX_COLS

    nds12 = np.zeros((P, SLICES_CORE), dtype=np.float32)
    nd3 = np.zeros((P, SLICES_CORE), dtype=np.float32)
    base = core * PER_CORE
    mask = (perm >= base) & (perm < base + PER_CORE)
    local = perm[mask] - base
    nds12[local % P, local // P] = (nd * ns)[mask]
    nd3[local % P, local // P] = nd[mask]
    return idx_all, rel_cols, nds12, nd3


def _build_program():
    nc = bacc.Bacc(
        "TRN2",
        target_bir_lowering=False,
        debug=False,
        num_devices=NCORES,
        num_swdge_queues=4,
    )
    dt = mybir.dt

    xn0_in = nc.declare_dram_parameter("xn0", [N_PAD, F_IN], dt.bfloat16, isOutput=False)
    idx_in = nc.declare_dram_parameter("idx", [P, IDX_COLS], dt.int16, isOutput=False)
    rel_in = nc.declare_dram_parameter("rel", [P, N_TILE_COLS], dt.bfloat16, isOutput=False)
    iota_in = nc.declare_dram_parameter("iota", [P, P], dt.bfloat16, isOutput=False)
    w1_in = nc.declare_dram_parameter("w1", [F_IN, F_HID], dt.bfloat16, isOutput=False)
    w2_in = nc.declare_dram_parameter("w2", [F_HID, F_HID], dt.bfloat16, isOutput=False)
    w3_in = nc.declare_dram_parameter("w3", [F_HID, F_OUT], dt.bfloat16, isOutput=False)
    b3_in = nc.declare_dram_parameter("b3rep", [P, F_OUT], dt.float32, isOutput=False)
    nds12_in = nc.declare_dram_parameter("nds12", [P, SLICES_CORE], dt.float32, isOutput=False)
    nd3_in = nc.declare_dram_parameter("nd3", [P, SLICES_CORE], dt.float32, isOutput=False)
    out_ext = nc.declare_dram_parameter("out", [PER_CORE, F_OUT], dt.float32, isOutput=True)

    with tile.TileContext(nc) as tc:
        with (
            tc.tile_pool(name="consts", bufs=1) as consts,
            tc.tile_pool(name="gt", bufs=2) as gtp,
            tc.tile_pool(name="work", bufs=4) as work,
            tc.tile_pool(name="outw", bufs=3) as outw,
            tc.tile_pool(name="psum", bufs=6, space="PSUM") as psum,
            tc.tile_pool(name="psw", bufs=2, space="PSUM") as psw,
            tc.tile_pool(name="dram", bufs=1, space="DRAM") as dram,
        ):
            nc.gpsimd.load_library(library_config.mlp)

            idx_t = consts.tile([P, IDX_COLS], dt.int16)
            rel_t = consts.tile([P, N_TILE_COLS], dt.bfloat16)
            iota_t = consts.tile([P, P], dt.bfloat16)
            w1_t = consts.tile([F_IN, F_HID], dt.bfloat16)
            w2_t = consts.tile([F_HID, F_HID], dt.bfloat16)
            w3_t = consts.tile([F_HID, F_OUT], dt.bfloat16)
            b3_t = consts.tile([P, F_OUT], dt.float32)
            nds12_t = consts.tile([P, SLICES_CORE], dt.float32)
            nd3_t = consts.tile([P, SLICES_CORE], dt.float32)
            nc.sync.dma_start(out=idx_t[:], in_=idx_in[:])
            nc.sync.dma_start(out=rel_t[:], in_=rel_in[:])
            nc.sync.dma_start(out=iota_t[:], in_=iota_in[:])
            nc.sync.dma_start(out=w1_t[:], in_=w1_in[:])
            nc.sync.dma_start(out=w2_t[:], in_=w2_in[:])
            nc.sync.dma_start(out=w3_t[:], in_=w3_in[:])
            nc.sync.dma_start(out=b3_t[:], in_=b3_in[:])
            nc.sync.dma_start(out=nds12_t[:], in_=nds12_in[:])
            nc.sync.dma_start(out=nd3_t[:], in_=nd3_in[:])

            ag_in1 = dram.tile([PER_CORE, F_HID], dt.bfloat16, tag="ag_in1")
            ag_in2 = dram.tile([PER_CORE, F_HID], dt.bfloat16, tag="ag_in2")
            xn1 = [
                dram.tile(
                    [CHUNK_ROWS, F_HID],
                    dt.bfloat16,
                    tag=f"xn1_{k}",
                    name=f"xn1_{k}",
                    addr_space="Shared",
                )
                for k in range(NCHUNK)
            ]
            xn2 = [
                dram.tile(
                    [CHUNK_ROWS, F_HID],
                    dt.bfloat16,
                    tag=f"xn2_{k}",
                    name=f"xn2_{k}",
                    addr_space="Shared",
                )
                for k in range(NCHUNK)
            ]
            xn0 = [
                xn0_in[c * CHUNK_ROWS : (c + 1) * CHUNK_ROWS, :] for c in range(NCHUNK)
            ]

            layers = [
                (xn0, w1_t, F_HID, nds12_t, ag_in1, xn1),
                (xn1, w2_t, F_HID, nds12_t, ag_in2, xn2),
                (xn2, w3_t, F_OUT, nd3_t, None, None),
            ]

            tc._gnn = (gtp, work, outw, psum, psw, idx_t, rel_t, iota_t, b3_t, out_ext)
            _emit_layers(nc, tc, layers)
    nc.compile()
    return nc


def _emit_layers(nc, tc, layers):
    dt = mybir.dt
    gtp, work, outw, psum, psw, idx_t, rel_t, iota_t, b3_t, out_ext = tc._gnn
    for li, (table, w_t, fo, scale_t, ag_in, ag_out) in enumerate(layers):
        if li >= _DBG_LAYERS:
            break
        with nc.named_scope(f"layer{li + 1}"):
            icol = 0
            for g in range(NGROUP):
                if g >= _DBG_GROUPS:
                    break
                gts = []
                for c in range(NCHUNK):
                    gt = gtp.tile([P, GRP * TSC, P], dt.bfloat16, tag=f"gt{c}")
                    nc.gpsimd.dma_gather(
                        gt[:],
                        table[c],
                        idx_t[:, icol : icol + NI_CALL // 16],
                        NI_CALL,
                        NI_CALL,
                        P,
                        queue_num=c,
                    )
                    icol += NI_CALL // 16
                    gts.append(gt)
                for si in range(GRP):
                    s = g * GRP + si
                    s_big = work.tile([P, NBLK * P], dt.bfloat16, tag="s_big")
                    sb3 = s_big[:].rearrange("p (b f) -> p b f", b=NBLK)
                    in0 = iota_t[:].unsqueeze(1).to_broadcast([P, NBLK, P])
                    in1 = (
                        rel_t[:, s * NBLK : (s + 1) * NBLK]
                        .unsqueeze(2)
                        .to_broadcast([P, NBLK, P])
                    )
                    nc.vector.tensor_tensor(
                        out=sb3, in0=in0, in1=in1, op=mybir.AluOpType.is_equal
                    )
                    acc = psum.tile([P, P], dt.float32, space="PSUM", tag="acc")
                    for c in range(NCHUNK):
                        for t in range(TSC):
                            b = c * TSC + t
                            nc.tensor.matmul(
                                out=acc[:],
                                lhsT=gts[c][:, si * TSC + t, :],
                                rhs=s_big[:, b * P : (b + 1) * P],
                                start=(b == 0),
                                stop=(b == NBLK - 1),
                            )
                    aggT = work.tile([P, P], dt.bfloat16, tag="aggT")
                    nc.vector.tensor_copy(out=aggT[:], in_=acc[:])
                    op = psw.tile([P, fo], dt.float32, space="PSUM", tag="op")
                    nc.tensor.matmul(
                        out=op[:], lhsT=aggT[:], rhs=w_t[:], start=True, stop=True
                    )
                    if li < 2:
                        o = outw.tile([P, fo], dt.bfloat16, tag="o")
                        nc.scalar.activation(
                            out=o[:],
                            in_=op[:],
                            func=mybir.ActivationFunctionType.Relu,
                            scale=scale_t[:, s : s + 1],
                        )
                        nc.sync.dma_start(out=ag_in[s * P : (s + 1) * P, :], in_=o[:])
                    else:
                        o = outw.tile([P, fo], dt.float32, tag="o32")
                        nc.scalar.activation(
                            out=o[:],
                            in_=op[:],
                            func=mybir.ActivationFunctionType.Copy,
                            scale=scale_t[:, s : s + 1],
                        )
                        nc.vector.tensor_add(out=o[:], in0=o[:], in1=b3_t[:])
                        nc.sync.dma_start(
                            out=out_ext[s * P : (s + 1) * P, :], in_=o[:]
                        )
                # Sub-AllGather k covers local slices [30k, 30k+30): emit as
                # soon as the 3 groups producing that quartile are done.
                if ag_in is not None and not _DBG_NO_AG:
                    ends = {(k + 1) * QUART // GRP - 1: k for k in range(NCHUNK)}
                    if g in ends:
                        k = ends[g]
                        nc.gpsimd.collective_compute(
                            "AllGather",
                            mybir.AluOpType.bypass,
                            replica_groups=[list(range(NCORES))],
                            ins=[ag_in[k * QUART * P : (k + 1) * QUART * P, :].opt()],
                            outs=[ag_out[k].opt()],
                        )


def _make_in_maps(x, src, dst, W1, W2, W3, b3):
    perm, trow, ns, nd, idx_pad, rel_pad = _preprocess(src, dst)

    xn0 = np.zeros((N_PAD, F_IN), dtype=BF16)
    xn0[trow] = (x * ns[:, None]).astype(BF16)
    iota = np.broadcast_to(np.arange(P, dtype=np.float32), (P, P)).astype(BF16).copy()
    b3rep = np.broadcast_to(b3, (P, F_OUT)).astype(np.float32).copy()

    in_maps = []
    for c in range(NCORES):
        idx_all, rel_cols, nds12, nd3 = _core_arrays(c, perm, ns, nd, idx_pad, rel_pad)
        in_maps.append(
            {
                "xn0": xn0,
                "idx": idx_all,
                "rel": rel_cols,
                "iota": iota,
                "w1": W1.astype(BF16),
                "w2": W2.astype(BF16),
                "w3": W3.astype(BF16),
                "b3rep": b3rep,
                "nds12": nds12,
                "nd3": nd3,
            }
        )
    return in_maps, perm


def kernel(x, src, dst, W1, W2, W3, b3):
    x = np.ascontiguousarray(np.asarray(x, dtype=np.float32))
    src = np.asarray(src).astype(np.int64)
    dst = np.asarray(dst).astype(np.int64)
    W1 = np.ascontiguousarray(np.asarray(W1, dtype=np.float32))
    W2 = np.ascontiguousarray(np.asarray(W2, dtype=np.float32))
    W3 = np.ascontiguousarray(np.asarray(W3, dtype=np.float32))
    b3 = np.ascontiguousarray(np.asarray(b3, dtype=np.float32))

    in_maps, perm = _make_in_maps(x, src, dst, W1, W2, W3, b3)
    nc = _build_program()
    res = run_bass_kernel_spmd(nc, in_maps, list(range(NCORES)))
    global LAST_RESULT
    LAST_RESULT = res

    full = np.concatenate([res.results[c]["out"] for c in range(NCORES)], axis=0)
    return full[perm].astype(np.float32)


LAST_RESULT = None
